# revision 1
# baseline (speedup 1.0000x reference)
"""Trainium2 Bass kernel: batched soft 3-SAT circuit evaluation.

out[b, c] = 1 - prod_k z[c,k],  z = (sign>0 ? 1-x : x)[idx],
x = sigmoid(emb[0]).  Every batch row is identical (input_idx is all
zeros, the embedding has a single row, and jnp.take clamps OOB), so the
device computes each clause result once and broadcast-writes the rows.

Sharding: clauses split across 8 NeuronCores (5250 each, padded 5376).
Host work is index-layout prep only (fold sign into a combined table
index, pad, order literals chunk-major, wrap into the 16-partition
GPSIMD gather layout) plus concatenation of per-core outputs.

Per-core device pipeline (H = 4 column chunks of 1344 cols):
  prologue (4 col-quarters, two HWDGE rings): broadcast-load emb row
    into raw[128, NV]; ACT sigmoid -> x table half; DVE (x*-1)+1 ->
    1-x table half.  Combined table tab[128, 2*NV].
  per chunk h:
    - GPSIMD ap_gather: z[128, 512] literals (8 Q7 groups x 168 clauses)
    - DVE: r = 1 - z0*z1*z2  [128, 168] (replicated within each
      16-partition group)
    - PE: per group g a [K=16]x[M=128]x[N=168] matmul with lhsT=1/16
      broadcasts group g's row into all 128 partitions of PSUM (bitwise
      exact: sum of 16 identical values * 1/16)
    - ACT: copy PSUM -> SBUF bcast tile [128, 8*168]
    - 8 row-block DMAs bcast -> out[128b:128b+128, 1344h:1344h+1344]
      (5.4KB descriptors), alternating the sync/scalar HWDGE rings.
"""

import numpy as np

NV = 10000
C_TOTAL = 42000
KLIT = 3
B = 1024
NCORES = 8
C_CORE = C_TOTAL // NCORES     # 5250
GROUPS = 8                     # Q7 cores / 16-partition groups
C_PAD = 5376                   # padded clauses per core
CPGS = [168, 168, 168, 84]     # clauses per (group, Q7-chunk)
H = len(CPGS)
C_CHUNKS = [8 * c for c in CPGS]          # output cols per Q7 chunk
C_OFFS = [sum(C_CHUNKS[:h]) for h in range(H)]
LPCS = [c * KLIT for c in CPGS]           # real literals per (g, chunk)
LPC_PADS = [-(-l // 32) * 32 for l in LPCS]   # pad to 32 (2-col align)
COLS_HS = [l // 16 for l in LPC_PADS]     # idx cols per chunk
COL_OFFS = [sum(COLS_HS[:h]) for h in range(H)]
IDX_COLS = sum(COLS_HS)
PBLK = 256                     # PSUM cols reserved per group block

# PE-gathered tail: the last 672 output cols are gathered on the tensor
# engine via one-hot radix matmuls while the Q7 cores work the rest.
PE_C = C_PAD - sum(C_CHUNKS)   # 672 clauses
PE_OFF = sum(C_CHUNKS)         # col offset 4704
PE_L = PE_C * KLIT             # 2016 literals
PE_LP = 2048                   # padded to 4 tiles of 512
PE_TILES = PE_LP // 512
RADIX = 128                    # idx' = 128*hi + lo; hi < 157, lo < 128

_CACHE = {}


def _build():
    import concourse.bass as bass
    import concourse.tile as tile
    from concourse import bacc, mybir
    from contextlib import ExitStack

    f32 = mybir.dt.float32
    AF = mybir.ActivationFunctionType
    OP = mybir.AluOpType

    nc = bacc.Bacc("TRN2", target_bir_lowering=False, debug=False,
                   num_devices=NCORES)
    emb_d = nc.dram_tensor("emb", [1, NV], f32, kind="ExternalInput")
    idx_d = nc.dram_tensor("idxw", [128, IDX_COLS], mybir.dt.int16,
                           kind="ExternalInput")
    hia_d = nc.dram_tensor("hia", [1, PE_LP], f32, kind="ExternalInput")
    hib_d = nc.dram_tensor("hib", [1, PE_LP], f32, kind="ExternalInput")
    lo_d = nc.dram_tensor("lo", [1, PE_LP], f32, kind="ExternalInput")
    out_d = nc.dram_tensor("out", [B, C_PAD], f32, kind="ExternalOutput")

    with tile.TileContext(nc) as tc, ExitStack() as ctx:
        const = ctx.enter_context(tc.tile_pool(name="const", bufs=1))
        work = ctx.enter_context(tc.tile_pool(name="work", bufs=2))
        psum = ctx.enter_context(
            tc.tile_pool(name="psum", bufs=1, space="PSUM"))
        pepsum = ctx.enter_context(
            tc.tile_pool(name="pepsum", bufs=2, space="PSUM"))
        dpool = ctx.enter_context(
            tc.tile_pool(name="dram", bufs=1, space="DRAM"))

        idx_sb = const.tile([128, IDX_COLS], mybir.dt.int16)

        # selector E[:, g, :]: E[k, g, m] = 1/16 iff k//16 == g; matmul
        # with it averages each group's 16 identical partition rows into
        # all 128 output partitions (bitwise exact).
        sel = const.tile([128, GROUPS, 128], f32)
        nc.vector.memset(sel[:], 1.0 / 16.0)
        # keep 1/16 only where 0 <= p - 16g <= 15, i.e. g == p//16
        nc.gpsimd.affine_select(sel[:, :, :], sel[:, :, :],
                                pattern=[[-16, GROUPS], [0, 128]],
                                compare_op=OP.is_ge, fill=0.0,
                                base=0, channel_multiplier=1)
        nc.gpsimd.affine_select(sel[:, :, :], sel[:, :, :],
                                pattern=[[16, GROUPS], [0, 128]],
                                compare_op=OP.is_ge, fill=0.0,
                                base=15, channel_multiplier=-1)

        # table padded to RADIX*157 = 20096 so the PE radix view is in
        # bounds; tail memset keeps the X2 copy finite
        tab = const.tile([128, 157 * RADIX], f32)
        nc.vector.memset(tab[:, 2 * NV:157 * RADIX], 0.0)
        rings = [nc.sync, nc.scalar]
        NQ = 8
        q = NV // NQ
        with tc.tile_pool(name="rawp", bufs=1) as rawp:
            raw = rawp.tile([128, NV], f32)
            # broadcast-load eighths alternate sync HWDGE / gpsimd
            # SWDGE: two queues give aggregate HBM-read rate, and the
            # scalar ring stays clear so ACT isn't delayed by dispatch
            for c in range(NQ):
                eng = nc.sync if c % 2 == 0 else nc.gpsimd
                eng.dma_start(
                    out=raw[:, c * q:(c + 1) * q],
                    in_=bass.AP(tensor=emb_d, offset=c * q,
                                ap=[[0, 128], [1, q]]))
            nc.gpsimd.dma_start(out=idx_sb[:], in_=idx_d[:, :])
            for c in range(NQ):
                sl = slice(c * q, (c + 1) * q)
                xs = slice(NV + c * q, NV + (c + 1) * q)
                nc.scalar.activation(tab[:, xs], raw[:, sl], AF.Sigmoid)
                # 1 - x on DVE, overlaps ACT of the next eighth
                nc.vector.tensor_scalar(tab[:, sl], tab[:, xs], -1.0,
                                        1.0, OP.mult, OP.add)

        # ---- PE-gather tail: one-hot inputs and table radix view ----
        hi_bc = const.tile([128, PE_LP], f32)
        hib_bc = const.tile([128, PE_LP], f32)
        lo_bc = const.tile([128, PE_LP], f32)
        for src_d, dst in ((hia_d, hi_bc), (hib_d, hib_bc), (lo_d, lo_bc)):
            nc.gpsimd.dma_start(
                out=dst[:],
                in_=bass.AP(tensor=src_d, offset=0,
                            ap=[[0, 128], [1, PE_LP]]))
        iota_i = const.tile([128, 1], mybir.dt.int32)
        nc.gpsimd.iota(iota_i[:], pattern=[[0, 1]], channel_multiplier=1)
        iota_f = const.tile([128, 1], f32)
        nc.vector.tensor_copy(iota_f[:], iota_i[:])
        ones_col = const.tile([128, 1], f32)
        nc.vector.memset(ones_col[:], 1.0)
        # one-hot masks per 512-literal tile (DVE, pre-gather window)
        oh_a, oh_b, oh_l = [], [], []
        for t in range(PE_TILES):
            sl = slice(512 * t, 512 * (t + 1))
            oa = const.tile([128, 512], f32, tag=f"oha{t}")
            nc.vector.tensor_scalar(oa[:], hi_bc[:, sl], iota_f[:, 0:1],
                                    None, OP.is_equal)
            ob = const.tile([128, 512], f32, tag=f"ohb{t}")
            nc.vector.tensor_scalar(ob[:], hib_bc[:, sl], iota_f[:, 0:1],
                                    None, OP.is_equal)
            ol = const.tile([128, 512], f32, tag=f"ohl{t}")
            nc.vector.tensor_scalar(ol[:], lo_bc[:, sl], iota_f[:, 0:1],
                                    None, OP.is_equal)
            oh_a.append(oa); oh_b.append(ob); oh_l.append(ol)
        # X2[k, m] = tab[128k + m] laid out across partitions
        x2a = const.tile([128, RADIX], f32)
        x2b = const.tile([29, RADIX], f32)
        tapr = tab[:].ap[0][0]
        nc.sync.dma_start(
            out=x2a[:],
            in_=bass.AP(tensor=tab[:].tensor, offset=tab[:].offset,
                        ap=[[tapr, 1], [1, 128 * RADIX]]))
        nc.sync.dma_start(
            out=x2b[:],
            in_=bass.AP(tensor=tab[:].tensor,
                        offset=tab[:].offset + 128 * RADIX,
                        ap=[[tapr, 1], [1, 29 * RADIX]]))
        # stage 1+2: Y = X2.T @ onehot_hi ; z = sum_p(Y * onehot_lo)
        zrow = const.tile([1, PE_LP], f32)
        for t in range(PE_TILES):
            Y = pepsum.tile([128, 512], f32, tag="Y")
            nc.tensor.matmul(Y[:], x2a[:], oh_a[t][:],
                             start=True, stop=False)
            nc.tensor.matmul(Y[:], x2b[:], oh_b[t][0:29, :],
                             start=False, stop=True)
            m_sb = work.tile([128, 512], f32, tag="msb")
            nc.vector.tensor_tensor(m_sb[:], Y[:], oh_l[t][:], OP.mult)
            zr = pepsum.tile([1, 512], f32, tag="zr")
            nc.tensor.matmul(zr[0:1, :], ones_col[:], m_sb[:],
                             start=True, stop=True)
            nc.scalar.activation(zrow[0:1, 512 * t:512 * (t + 1)],
                                 zr[0:1, :], AF.Copy)
        # products + (1 - .) on the single-partition row
        perow = const.tile([1, PE_C], f32)
        nc.vector.tensor_tensor(perow[0:1, :], zrow[0:1, 0:PE_L:3],
                                zrow[0:1, 1:PE_L:3], OP.mult)
        nc.vector.scalar_tensor_tensor(perow[0:1, :], perow[0:1, :], 1.0,
                                       zrow[0:1, 2:PE_L:3],
                                       OP.mult, OP.mult)
        nc.vector.tensor_scalar(perow[0:1, :], perow[0:1, :], -1.0, 1.0,
                                OP.mult, OP.add)
        # roundtrip through DRAM to broadcast across partitions
        drow = dpool.tile([1, PE_C], f32)
        nc.scalar.dma_start(out=drow[0:1, :], in_=perow[0:1, :])

        for h in range(H):
            CPG, LPC, LPC_PAD = CPGS[h], LPCS[h], LPC_PADS[h]
            C_CHUNK, C_OFF = C_CHUNKS[h], C_OFFS[h]
            z = work.tile([128, max(LPC_PADS)], f32, tag="z")
            nc.gpsimd.ap_gather(
                z[:, 0:LPC_PAD], tab[:],
                idx_sb[:, COL_OFFS[h]:COL_OFFS[h] + COLS_HS[h]],
                channels=128, num_elems=2 * NV, d=1, num_idxs=LPC_PAD)

            p01 = work.tile([128, max(CPGS)], f32, tag="p01")
            nc.vector.tensor_tensor(p01[:, 0:CPG], z[:, 0:LPC:3],
                                    z[:, 1:LPC:3], OP.mult)
            r = work.tile([128, max(CPGS)], f32, tag="r")
            # r = z0 z1 z2 (the 1 - . fold happens in the ACT copy)
            nc.vector.scalar_tensor_tensor(r[:, 0:CPG], p01[:, 0:CPG],
                                           1.0, z[:, 2:LPC:3],
                                           OP.mult, OP.mult)

            # PE broadcast: group g's (16-replicated) row -> all 128
            # partitions.  sum over the 16 identical values * 1/16 is
            # bitwise exact.
            P = psum.tile([128, GROUPS, PBLK], f32, tag="P")
            for g in range(GROUPS):
                nc.tensor.matmul(P[:, g, 0:CPG], sel[:, g, :],
                                 r[:, 0:CPG], start=True, stop=True)
            # pack the 8 group blocks contiguously so output descriptors
            # are C_CHUNK*4 bytes
            bcast = work.tile([128, GROUPS * max(CPGS)], f32, tag="bcast")
            bt = bcast[:]
            prow = bt.ap[0][0]
            bview = bass.AP(tensor=bt.tensor, offset=bt.offset,
                            ap=[[prow, 128], [CPG, GROUPS], [1, CPG]])
            # bcast = Copy(-P + 1) = 1 - z0 z1 z2
            nc.scalar.activation(bview, P[:, :, 0:CPG], AF.Copy,
                                 scale=-1.0, bias=1.0)

            out_w = C_CHUNK
            if h == H - 1:
                # append the PE-gathered tail columns via a stride-0
                # broadcast read of the DRAM row
                peb = bass.AP(tensor=bt.tensor, offset=bt.offset + C_CHUNK,
                              ap=[[prow, 128], [1, PE_C]])
                dr = drow[0:1, :]
                nc.scalar.dma_start(
                    out=peb,
                    in_=bass.AP(tensor=dr.tensor, offset=dr.offset,
                                ap=[[0, 128], [1, PE_C]]))
                out_w = C_CHUNK + PE_C

            # 8 row-block output DMAs, 128 rows each, spread across both
            # HWDGE rings
            bap = bass.AP(tensor=bt.tensor, offset=bt.offset,
                          ap=[[prow, 128], [1, out_w]])
            for blk in range(8):
                dst = bass.AP(tensor=out_d,
                              offset=blk * 128 * C_PAD + C_OFF,
                              ap=[[C_PAD, 128], [1, out_w]])
                rings[blk % 2].dma_start(out=dst, in_=bap)
    nc.compile()
    return nc


def _prep_indices(clause_idx, clause_sign):
    """Per-core wrapped int16 combined-index arrays [128, IDX_COLS].

    Literal order per group g: chunk-major — for chunk h, group g owns
    core clauses [C_CHUNK*h + CPG*g, C_CHUNK*h + CPG*(g+1)), padded to
    LPC_PAD literals per (group, chunk) block.
    """
    idx2 = clause_idx.astype(np.int32) + NV * (clause_sign <= 0.0)
    idx2 = idx2.astype(np.int16)
    per_core = []
    for c in range(NCORES):
        cl = idx2[c * C_CORE:(c + 1) * C_CORE]            # [5250, 3]
        buf = np.zeros((C_PAD, KLIT), dtype=np.int16)
        buf[:cl.shape[0]] = cl
        # group g's stream = concat over chunks of its padded block
        gs = np.zeros((GROUPS, IDX_COLS * 16), dtype=np.int16)
        for h in range(H):
            blk = buf[C_OFFS[h]:C_OFFS[h] + C_CHUNKS[h]]  # [8*CPG, 3]
            blk = blk.reshape(GROUPS, LPCS[h])
            o = COL_OFFS[h] * 16
            gs[:, o:o + LPCS[h]] = blk
        # wrap: literal j at partition 16g + j%16, col j//16
        w = (gs.reshape(GROUPS, IDX_COLS, 16)
               .transpose(0, 2, 1)
               .reshape(128, IDX_COLS))
        # PE tail: radix-decomposed literals, plain order, f32 rows
        pe = buf[PE_OFF:PE_OFF + PE_C].reshape(-1).astype(np.int32)
        pe = np.concatenate([pe, np.zeros(PE_LP - PE_L, np.int32)])
        hi = pe // RADIX
        hia = hi.astype(np.float32)[None, :]
        hib = (hi - 128).astype(np.float32)[None, :]
        lo = (pe % RADIX).astype(np.float32)[None, :]
        per_core.append((np.ascontiguousarray(w), hia, hib, lo))
    return per_core


def _ensure_ntff_hook():
    """The agent image lacks antenv.axon_hooks; synthesize it so
    run_bass_kernel_spmd(trace=True) can capture NTFF profiles."""
    import sys, types
    try:
        from antenv import axon_hooks  # noqa: F401
        return
    except ImportError:
        pass
    m = types.ModuleType("antenv.axon_hooks")
    _hook = [None]
    m.set_axon_ntff_profile_hook = lambda h: _hook.__setitem__(0, h)
    m.get_axon_ntff_profile_hook = lambda: _hook[0]
    sys.modules["antenv.axon_hooks"] = m
    import antenv
    antenv.axon_hooks = m
    from trn_agent_boot.trn_boot import _ntff_profile_via_ctypes
    m.set_axon_ntff_profile_hook(
        _ntff_profile_via_ctypes("/opt/axon/libaxon_pjrt.so"))


def _run(emb, idx_cores, trace=False):
    from concourse.bass_utils import run_bass_kernel_spmd
    if trace:
        _ensure_ntff_hook()
    if "prog" not in _CACHE:
        _CACHE["prog"] = _build()
    nc = _CACHE["prog"]
    in_maps = [{"emb": emb, "idxw": idx_cores[c][0],
                "hia": idx_cores[c][1], "hib": idx_cores[c][2],
                "lo": idx_cores[c][3]} for c in range(NCORES)]
    return run_bass_kernel_spmd(nc, in_maps, list(range(NCORES)),
                                trace=trace)


def kernel(input_idx=None, emb_weight=None, clause_idx=None,
           clause_sign=None, _trace=False, _want_results=False):
    emb = np.ascontiguousarray(np.asarray(emb_weight, dtype=np.float32))
    cidx = np.asarray(clause_idx, dtype=np.int32)
    csgn = np.asarray(clause_sign, dtype=np.float32)
    idx_cores = _prep_indices(cidx, csgn)
    res = _run(emb, idx_cores, trace=_trace)
    full = np.empty((B, C_TOTAL), dtype=np.float32)
    for c in range(NCORES):
        full[:, c * C_CORE:(c + 1) * C_CORE] = \
            res.results[c]["out"][:, :C_CORE]
    if _want_results:
        return full, res
    return full



# revision 8
# speedup vs baseline: 1.2347x; 1.2347x over previous
"""Trainium2 Bass kernel: batched soft 3-SAT circuit evaluation.

out[b, c] = 1 - prod_k z[c,k],  z = (sign>0 ? 1-x : x)[idx],
x = sigmoid(emb[0]).  Every batch row is identical (input_idx is all
zeros, the embedding has a single row, and jnp.take clamps OOB), so the
device computes each clause result once and broadcast-writes the rows.

Sharding: clauses split across 8 NeuronCores (5250 each, padded 5376).
Host work is layout prep only: fold sign into a combined table index,
pad, wrap into the 16-partition GPSIMD gather layout, and narrow the
embedding row to fp16.  Output is written as fp16 (tolerance 2e-2;
fp16 error ~1e-3) and upcast to f32 on the host, halving the dominant
HBM write (22 MB -> 11 MB per core).

Per-core device pipeline (H = 4 column chunks of 1344 cols):
  prologue: warmup sigmoid (preloads the ACT table), idx DMA, then 8
    column-eighths: broadcast-load the fp16 emb row into raw16[128,NV]
    (sync HWDGE / gpsimd SWDGE alternating), ACT sigmoid (fp16 in,
    f32 out) -> x half of tab, DVE (x*-1)+1 -> 1-x half.
  per chunk h:
    - GPSIMD ap_gather: z[128, 512] literals (8 Q7 groups x 168
      clauses, 16x replicated within each group)
    - DVE: r = z0*z1*z2  [128, 168]
    - PE: per group g a K=1 outer product (ones row x r row) broadcasts
      group g's row into all 128 PSUM partitions (bitwise exact)
    - ACT/DVE (alternating): bcast[:, chunk] = 1 - P, fp16
  out: after chunks {0,1} and {2,3}, two row-half DMAs each
    (512 rows x 2688/2562 cols, ~5.3KB descriptors) on the sync and
    scalar HWDGE rings.
"""

import numpy as np

NV = 10000
C_TOTAL = 42000
KLIT = 3
B = 1024
NCORES = 8
C_CORE = C_TOTAL // NCORES     # 5250
GROUPS = 8                     # Q7 cores / 16-partition groups
H = 4                          # literal chunks
CPG = 168                      # clauses per (group, chunk)
C_CHUNK = GROUPS * CPG         # 1344 output cols per chunk
C_PAD = H * C_CHUNK            # 5376
LPC = CPG * KLIT               # 504 real literals per (group, chunk)
LPC_PAD = 512                  # padded to 32 idx cols
COLS_H = LPC_PAD // 16         # 32 idx cols per chunk
IDX_COLS = H * COLS_H          # 128
PBLK = 256                     # PSUM cols reserved per group block
NQ = 8                         # emb broadcast-load eighths
QW = NV // NQ                  # 1250
# column pairs for the output DMAs: chunks {0,1} then {2,3}
PAIR_OFF = [0, 2 * C_CHUNK]
PAIR_W = [2 * C_CHUNK, C_CORE - 2 * C_CHUNK]   # 2688, 2562

_CACHE = {}


def _build():
    import concourse.bass as bass
    import concourse.tile as tile
    from concourse import bacc, mybir
    from contextlib import ExitStack

    f32 = mybir.dt.float32
    f16 = mybir.dt.float16
    AF = mybir.ActivationFunctionType
    OP = mybir.AluOpType

    nc = bacc.Bacc("TRN2", target_bir_lowering=False, debug=False,
                   num_devices=NCORES)
    emb_d = nc.dram_tensor("emb16", [1, NV], f16, kind="ExternalInput")
    idx_d = nc.dram_tensor("idxw", [128, IDX_COLS], mybir.dt.int16,
                           kind="ExternalInput")
    selm_d = nc.dram_tensor("selm", [128, 4, 128], f32,
                            kind="ExternalInput")
    out_d = nc.dram_tensor("out", [B, C_CORE], f16, kind="ExternalOutput")

    with tile.TileContext(nc) as tc, ExitStack() as ctx:
        const = ctx.enter_context(tc.tile_pool(name="const", bufs=1))
        work = ctx.enter_context(tc.tile_pool(name="work", bufs=2))
        psum = ctx.enter_context(
            tc.tile_pool(name="psum", bufs=2, space="PSUM"))

        # warmup: preload the ACT sigmoid table while the first DMA is
        # in flight
        warm = const.tile([128, 8], f32)
        nc.vector.memset(warm[:], 0.0)
        nc.scalar.activation(warm[:], warm[:], AF.Sigmoid)

        # mask weights for the K=64 PE broadcast: selm[p, j, :] = 1/16
        # iff (p%64)//16 == j (PE tiles require base partition 0/32/64,
        # so each matmul spans four groups and the mask picks one)
        selm = const.tile([128, 4, 128], f32)
        nc.gpsimd.dma_start(out=selm[:], in_=selm_d[:, :, :])

        idx_sb = const.tile([128, IDX_COLS], mybir.dt.int16)
        nc.gpsimd.dma_start(out=idx_sb[:], in_=idx_d[:, :])

        # combined table: [0:NV) = 1-x (positive literals),
        # [NV:2NV) = x (negative literals)
        tab = const.tile([128, 2 * NV], f32)
        raw = const.tile([128, NV], f16)
        for c in range(NQ):
            eng = nc.sync if c % 2 == 0 else nc.gpsimd
            eng.dma_start(
                out=raw[:, c * QW:(c + 1) * QW],
                in_=bass.AP(tensor=emb_d, offset=c * QW,
                            ap=[[0, 128], [1, QW]]))
        for c in range(NQ):
            sl = slice(c * QW, (c + 1) * QW)
            xs = slice(NV + c * QW, NV + (c + 1) * QW)
            nc.scalar.activation(tab[:, xs], raw[:, sl], AF.Sigmoid)
            nc.vector.tensor_scalar(tab[:, sl], tab[:, xs], -1.0, 1.0,
                                    OP.mult, OP.add)

        # single full-width staging tile for the broadcast rows
        bcast = const.tile([128, C_PAD], f16)
        bt = bcast[:]
        prow = bt.ap[0][0]

        copy_engs = [nc.scalar, nc.vector, nc.scalar, nc.vector]
        for h in range(H):
            z = work.tile([128, LPC_PAD], f32, tag="z")
            nc.gpsimd.ap_gather(
                z[:, 0:LPC_PAD], tab[:],
                idx_sb[:, COLS_H * h:COLS_H * (h + 1)],
                channels=128, num_elems=2 * NV, d=1, num_idxs=LPC_PAD)

            p01 = work.tile([128, CPG], f32, tag="p01")
            nc.vector.tensor_tensor(p01[:, 0:CPG], z[:, 0:LPC:3],
                                    z[:, 1:LPC:3], OP.mult)
            r = work.tile([128, CPG], f32, tag="r")
            # r = z0 z1 z2 (the 1 - . fold happens in the copy below)
            nc.vector.scalar_tensor_tensor(r[:, 0:CPG], p01[:, 0:CPG],
                                           1.0, z[:, 2:LPC:3],
                                           OP.mult, OP.mult)

            # K=32 masked broadcast: group g's (16-replicated) row -> all
            # 128 PSUM partitions; sum of 16 identical values * 1/16 is
            # bitwise exact
            P = psum.tile([128, GROUPS, PBLK], f32, tag="P")
            for g in range(GROUPS):
                base = 64 * (g // 4)
                nc.tensor.matmul(P[:, g, 0:CPG],
                                 selm[base:base + 64, g % 4, :],
                                 r[base:base + 64, 0:CPG],
                                 start=True, stop=True)
            # pack the 8 group blocks contiguously: bcast = 1 - P, fp16
            bview = bass.AP(tensor=bt.tensor,
                            offset=bt.offset + h * C_CHUNK,
                            ap=[[prow, 128], [CPG, GROUPS], [1, CPG]])
            eng = copy_engs[h]
            if eng is nc.scalar:
                eng.activation(bview, P[:, :, 0:CPG], AF.Copy,
                               scale=-1.0, bias=1.0)
            else:
                eng.tensor_scalar(bview, P[:, :, 0:CPG], -1.0, 1.0,
                                  OP.mult, OP.add)

            if h % 2 == 1:
                pair = h // 2
                off, w = PAIR_OFF[pair], PAIR_W[pair]
                # every bcast partition holds the same row, so the
                # src->dst row mapping is free; repeat each partition 4x
                src = bass.AP(tensor=bt.tensor, offset=bt.offset + off,
                              ap=[[prow, 128], [0, 4], [1, w]])
                for s, ring in enumerate((nc.sync, nc.scalar)):
                    dst = bass.AP(tensor=out_d,
                                  offset=s * 512 * C_CORE + off,
                                  ap=[[C_CORE, 512], [1, w]])
                    ring.dma_start(out=dst, in_=src)
    nc.compile()
    return nc


def _prep_indices(clause_idx, clause_sign):
    """Per-core wrapped int16 combined-index arrays [128, IDX_COLS].

    Chunk h, group g owns core clauses [C_CHUNK*h + CPG*g,
    C_CHUNK*h + CPG*(g+1)); its 504 literals (pad 512) are wrapped so
    literal j sits at partition 16g + j%16, col 32h + j//16.
    """
    idx2 = clause_idx.astype(np.int32) + NV * (clause_sign <= 0.0)
    idx2 = idx2.astype(np.int16)
    per_core = []
    for c in range(NCORES):
        cl = idx2[c * C_CORE:(c + 1) * C_CORE]            # [5250, 3]
        buf = np.zeros((C_PAD, KLIT), dtype=np.int16)
        buf[:cl.shape[0]] = cl
        s = buf.reshape(H, GROUPS, LPC)                   # [4, 8, 504]
        st = np.zeros((H, GROUPS, LPC_PAD), dtype=np.int16)
        st[:, :, :LPC] = s
        w = (st.reshape(H, GROUPS, COLS_H, 16)
               .transpose(1, 3, 0, 2)
               .reshape(128, IDX_COLS))
        per_core.append(np.ascontiguousarray(w))
    return per_core


def _ensure_ntff_hook():
    """The agent image lacks antenv.axon_hooks; synthesize it so
    run_bass_kernel_spmd(trace=True) can capture NTFF profiles."""
    import sys, types
    try:
        from antenv import axon_hooks  # noqa: F401
        return
    except ImportError:
        pass
    m = types.ModuleType("antenv.axon_hooks")
    _hook = [None]
    m.set_axon_ntff_profile_hook = lambda h: _hook.__setitem__(0, h)
    m.get_axon_ntff_profile_hook = lambda: _hook[0]
    sys.modules["antenv.axon_hooks"] = m
    import antenv
    antenv.axon_hooks = m
    from trn_agent_boot.trn_boot import _ntff_profile_via_ctypes
    m.set_axon_ntff_profile_hook(
        _ntff_profile_via_ctypes("/opt/axon/libaxon_pjrt.so"))


def _selm():
    p = np.arange(128)
    m = np.zeros((128, 4, 128), dtype=np.float32)
    for j in range(4):
        m[(p % 64) // 16 == j, j, :] = 1.0 / 16.0
    return m


def _run(emb16, idx_cores, trace=False):
    from concourse.bass_utils import run_bass_kernel_spmd
    if trace:
        _ensure_ntff_hook()
    if "prog" not in _CACHE:
        _CACHE["prog"] = _build()
    nc = _CACHE["prog"]
    selm = _selm()
    in_maps = [{"emb16": emb16, "idxw": idx_cores[c], "selm": selm}
               for c in range(NCORES)]
    return run_bass_kernel_spmd(nc, in_maps, list(range(NCORES)),
                                trace=trace)


def kernel(input_idx=None, emb_weight=None, clause_idx=None,
           clause_sign=None, _trace=False, _want_results=False):
    emb16 = np.ascontiguousarray(
        np.asarray(emb_weight, dtype=np.float32).astype(np.float16))
    cidx = np.asarray(clause_idx, dtype=np.int32)
    csgn = np.asarray(clause_sign, dtype=np.float32)
    idx_cores = _prep_indices(cidx, csgn)
    res = _run(emb16, idx_cores, trace=_trace)
    full = np.empty((B, C_TOTAL), dtype=np.float32)
    for c in range(NCORES):
        full[:, c * C_CORE:(c + 1) * C_CORE] = res.results[c]["out"]
    if _want_results:
        return full, res
    return full


# revision 9
# speedup vs baseline: 1.3212x; 1.0701x over previous
"""Trainium2 Bass kernel: batched soft 3-SAT circuit evaluation.

out[b, c] = 1 - prod_k z[c,k],  z = (sign>0 ? 1-x : x)[idx],
x = sigmoid(emb[0]).  Every batch row is identical (input_idx is all
zeros, the embedding has a single row, and jnp.take clamps OOB), so the
device computes each clause result once and broadcast-writes the rows.

Sharding: clauses split across 8 NeuronCores (5250 each, padded 5376).
Host work is layout prep only: fold sign into a combined table index,
pad, wrap into the 16-partition GPSIMD gather layout, and narrow the
embedding row to fp16.  Output is written as fp16 (tolerance 2e-2;
fp16 error ~1e-3) and upcast to f32 on the host, halving the dominant
HBM write (22 MB -> 11 MB per core).

The Q7 ap_gather costs ~28ns per index (per Q7 core, SIMD over its 16
partitions), so the gather of 672*3 literals per Q7 core (~57us burst
serial) is the pipeline's rate limiter.  The schedule therefore:
  - front-loads the ~16us ap_gather ucode library reload via
    load_library as the FIRST gpsimd instruction (it otherwise lands
    after the table build, serializing);
  - keeps every other prologue DMA on the sync HWDGE ring so the
    gpsimd engine is free;
  - uses descending chunk sizes [210,210,210,42] clauses/group so the
    final gather's output tail is small.

Per-core device pipeline:
  prologue: warmup sigmoid (preloads the ACT table), library preload,
    then 8 column-eighths: broadcast-load the fp16 emb row into
    raw[128,NV] (sync ring), ACT sigmoid (fp16 in, f32 out) -> x half
    of tab, DVE (x*-1)+1 -> 1-x half.
  per chunk h:
    - GPSIMD ap_gather: z[128, LPC_PAD] literals (8 Q7 groups x CPG
      clauses, 16x replicated within each group)
    - DVE: r = z0*z1*z2  [128, CPG]
    - PE: per group g a K=64 masked matmul (mask = 1/16 on the group's
      16 partitions) broadcasts the row into all 128 PSUM partitions
      (bitwise exact)
    - ACT/DVE (alternating): bcast[:, chunk] = 1 - P, fp16
    - 2 row-half DMAs (512 rows x chunk cols) on sync/scalar rings
"""

import numpy as np

NV = 10000
C_TOTAL = 42000
KLIT = 3
B = 1024
NCORES = 8
C_CORE = C_TOTAL // NCORES     # 5250
GROUPS = 8                     # Q7 cores / 16-partition groups
CPGS = [210, 210, 210, 42]     # clauses per (group, chunk)
H = len(CPGS)
C_CHUNKS = [GROUPS * c for c in CPGS]          # output cols per chunk
C_OFFS = [sum(C_CHUNKS[:h]) for h in range(H)]
C_PAD = sum(C_CHUNKS)          # 5376
LPCS = [c * KLIT for c in CPGS]                # real literals
LPC_PADS = [-(-l // 32) * 32 for l in LPCS]    # pad to 32 (2-col align)
COLS_HS = [l // 16 for l in LPC_PADS]          # idx cols per chunk
COL_OFFS = [sum(COLS_HS[:h]) for h in range(H)]
IDX_COLS = sum(COLS_HS)        # 128
PBLK = 256                     # PSUM cols reserved per group block
NQ = 8                         # emb broadcast-load eighths
QW = NV // NQ                  # 1250

_CACHE = {}


def _build():
    import concourse.bass as bass
    import concourse.tile as tile
    from concourse import bacc, mybir, library_config
    from contextlib import ExitStack

    f32 = mybir.dt.float32
    f16 = mybir.dt.float16
    AF = mybir.ActivationFunctionType
    OP = mybir.AluOpType

    nc = bacc.Bacc("TRN2", target_bir_lowering=False, debug=False,
                   num_devices=NCORES)
    emb_d = nc.dram_tensor("emb16", [1, NV], f16, kind="ExternalInput")
    idx_d = nc.dram_tensor("idxw", [128, IDX_COLS], mybir.dt.int16,
                           kind="ExternalInput")
    selm_d = nc.dram_tensor("selm", [128, 4, 128], f32,
                            kind="ExternalInput")
    out_d = nc.dram_tensor("out", [B, C_CORE], f16, kind="ExternalOutput")

    with tile.TileContext(nc) as tc, ExitStack() as ctx:
        const = ctx.enter_context(tc.tile_pool(name="const", bufs=1))
        work = ctx.enter_context(tc.tile_pool(name="work", bufs=2))
        psum = ctx.enter_context(
            tc.tile_pool(name="psum", bufs=2, space="PSUM"))

        # front-load the ~16us ap_gather Q7 ucode load; nothing else may
        # occupy the gpsimd engine before the first real gather
        nc.gpsimd.load_library(library_config.ap_gather)

        # warmup: preload the ACT sigmoid table while DMAs are in flight
        warm = const.tile([128, 8], f32)
        nc.vector.memset(warm[:], 0.0)
        nc.scalar.activation(warm[:], warm[:], AF.Sigmoid)

        # combined table: [0:NV) = 1-x (positive literals),
        # [NV:2NV) = x (negative literals)
        tab = const.tile([128, 2 * NV], f32)
        raw = const.tile([128, NV], f16)
        for c in range(NQ):
            nc.sync.dma_start(
                out=raw[:, c * QW:(c + 1) * QW],
                in_=bass.AP(tensor=emb_d, offset=c * QW,
                            ap=[[0, 128], [1, QW]]))

        idx_sb = const.tile([128, IDX_COLS], mybir.dt.int16)
        nc.sync.dma_start(out=idx_sb[:], in_=idx_d[:, :])
        # mask weights for the K=64 PE broadcast: selm[p, j, :] = 1/16
        # iff (p%64)//16 == j (PE tiles require base partition 0/32/64,
        # so each matmul spans four groups and the mask picks one)
        selm = const.tile([128, 4, 128], f32)
        nc.sync.dma_start(out=selm[:], in_=selm_d[:, :, :])

        for c in range(NQ):
            sl = slice(c * QW, (c + 1) * QW)
            xs = slice(NV + c * QW, NV + (c + 1) * QW)
            nc.scalar.activation(tab[:, xs], raw[:, sl], AF.Sigmoid)
            nc.vector.tensor_scalar(tab[:, sl], tab[:, xs], -1.0, 1.0,
                                    OP.mult, OP.add)

        # single full-width staging tile for the broadcast rows
        bcast = const.tile([128, C_PAD], f16)
        bt = bcast[:]
        prow = bt.ap[0][0]

        copy_engs = [nc.scalar, nc.vector, nc.scalar, nc.vector]
        for h in range(H):
            CPG, LPC, LPC_PAD = CPGS[h], LPCS[h], LPC_PADS[h]
            C_OFF = C_OFFS[h]
            z = work.tile([128, max(LPC_PADS)], f32, tag="z")
            nc.gpsimd.ap_gather(
                z[:, 0:LPC_PAD], tab[:],
                idx_sb[:, COL_OFFS[h]:COL_OFFS[h] + COLS_HS[h]],
                channels=128, num_elems=2 * NV, d=1, num_idxs=LPC_PAD)

            p01 = work.tile([128, max(CPGS)], f32, tag="p01")
            nc.vector.tensor_tensor(p01[:, 0:CPG], z[:, 0:LPC:3],
                                    z[:, 1:LPC:3], OP.mult)
            r = work.tile([128, max(CPGS)], f32, tag="r")
            # r = z0 z1 z2 (the 1 - . fold happens in the copy below)
            nc.vector.scalar_tensor_tensor(r[:, 0:CPG], p01[:, 0:CPG],
                                           1.0, z[:, 2:LPC:3],
                                           OP.mult, OP.mult)

            # K=64 masked broadcast: group g's (16-replicated) row -> all
            # 128 PSUM partitions; sum of 16 identical values * 1/16 is
            # bitwise exact
            P = psum.tile([128, GROUPS, PBLK], f32, tag="P")
            for g in range(GROUPS):
                base = 64 * (g // 4)
                nc.tensor.matmul(P[:, g, 0:CPG],
                                 selm[base:base + 64, g % 4, :],
                                 r[base:base + 64, 0:CPG],
                                 start=True, stop=True)
            # pack the 8 group blocks contiguously: bcast = 1 - P, fp16
            bview = bass.AP(tensor=bt.tensor,
                            offset=bt.offset + C_OFF,
                            ap=[[prow, 128], [CPG, GROUPS], [1, CPG]])
            eng = copy_engs[h]
            if eng is nc.scalar:
                eng.activation(bview, P[:, :, 0:CPG], AF.Copy,
                               scale=-1.0, bias=1.0)
            else:
                eng.tensor_scalar(bview, P[:, :, 0:CPG], -1.0, 1.0,
                                  OP.mult, OP.add)

            # 2 row-half output DMAs; last chunk trims the 126 pad cols
            w = min(C_CHUNKS[h], C_CORE - C_OFF)
            # every bcast partition holds the same row, so the src->dst
            # row mapping is free; repeat each partition 4x
            src = bass.AP(tensor=bt.tensor, offset=bt.offset + C_OFF,
                          ap=[[prow, 128], [0, 4], [1, w]])
            for s, ring in enumerate((nc.sync, nc.scalar)):
                dst = bass.AP(tensor=out_d,
                              offset=s * 512 * C_CORE + C_OFF,
                              ap=[[C_CORE, 512], [1, w]])
                ring.dma_start(out=dst, in_=src)
    nc.compile()
    return nc


def _prep_indices(clause_idx, clause_sign):
    """Per-core wrapped int16 combined-index arrays [128, IDX_COLS].

    Chunk h, group g owns core clauses [C_OFFS[h] + CPG*g, ...); its
    literals (padded to LPC_PAD) are wrapped so literal j sits at
    partition 16g + j%16, col COL_OFFS[h] + j//16.
    """
    idx2 = clause_idx.astype(np.int32) + NV * (clause_sign <= 0.0)
    idx2 = idx2.astype(np.int16)
    per_core = []
    for c in range(NCORES):
        cl = idx2[c * C_CORE:(c + 1) * C_CORE]            # [5250, 3]
        buf = np.zeros((C_PAD, KLIT), dtype=np.int16)
        buf[:cl.shape[0]] = cl
        w = np.zeros((128, IDX_COLS), dtype=np.int16)
        for h in range(H):
            blk = buf[C_OFFS[h]:C_OFFS[h] + C_CHUNKS[h]]  # [8*CPG, 3]
            st = np.zeros((GROUPS, LPC_PADS[h]), dtype=np.int16)
            st[:, :LPCS[h]] = blk.reshape(GROUPS, LPCS[h])
            # wrap: literal j at partition 16g + j%16, col j//16
            wh = (st.reshape(GROUPS, COLS_HS[h], 16)
                    .transpose(0, 2, 1)
                    .reshape(128, COLS_HS[h]))
            w[:, COL_OFFS[h]:COL_OFFS[h] + COLS_HS[h]] = wh
        per_core.append(np.ascontiguousarray(w))
    return per_core


def _selm():
    p = np.arange(128)
    m = np.zeros((128, 4, 128), dtype=np.float32)
    for j in range(4):
        m[(p % 64) // 16 == j, j, :] = 1.0 / 16.0
    return m


def _ensure_ntff_hook():
    """The agent image lacks antenv.axon_hooks; synthesize it so
    run_bass_kernel_spmd(trace=True) can capture NTFF profiles."""
    import sys, types
    try:
        from antenv import axon_hooks  # noqa: F401
        return
    except ImportError:
        pass
    m = types.ModuleType("antenv.axon_hooks")
    _hook = [None]
    m.set_axon_ntff_profile_hook = lambda h: _hook.__setitem__(0, h)
    m.get_axon_ntff_profile_hook = lambda: _hook[0]
    sys.modules["antenv.axon_hooks"] = m
    import antenv
    antenv.axon_hooks = m
    from trn_agent_boot.trn_boot import _ntff_profile_via_ctypes
    m.set_axon_ntff_profile_hook(
        _ntff_profile_via_ctypes("/opt/axon/libaxon_pjrt.so"))


def _run(emb16, idx_cores, trace=False):
    from concourse.bass_utils import run_bass_kernel_spmd
    if trace:
        _ensure_ntff_hook()
    if "prog" not in _CACHE:
        _CACHE["prog"] = _build()
    nc = _CACHE["prog"]
    selm = _selm()
    in_maps = [{"emb16": emb16, "idxw": idx_cores[c], "selm": selm}
               for c in range(NCORES)]
    return run_bass_kernel_spmd(nc, in_maps, list(range(NCORES)),
                                trace=trace)


def kernel(input_idx=None, emb_weight=None, clause_idx=None,
           clause_sign=None, _trace=False, _want_results=False):
    emb16 = np.ascontiguousarray(
        np.asarray(emb_weight, dtype=np.float32).astype(np.float16))
    cidx = np.asarray(clause_idx, dtype=np.int32)
    csgn = np.asarray(clause_sign, dtype=np.float32)
    idx_cores = _prep_indices(cidx, csgn)
    res = _run(emb16, idx_cores, trace=_trace)
    full = np.empty((B, C_TOTAL), dtype=np.float32)
    for c in range(NCORES):
        full[:, c * C_CORE:(c + 1) * C_CORE] = res.results[c]["out"]
    if _want_results:
        return full, res
    return full
